# revision 13
# baseline (speedup 1.0000x reference)
"""GAT EncodeProcessDecode (4 GAT layers) on 8 Trainium2 NeuronCores.

Strategy (graph/data parallel, per sharding hint):
  - Nodes are sharded contiguously across the 8 cores (dst-sharding).
  - Per layer, each core computes "augmented rows" [h | 1.0 | s_src | s_dst]
    for its local nodes with PE matmuls (the per-node attention scalars ride
    the same matmul via host-augmented weight matrices), then an AllGather
    replicates the full row table to every core.
  - The edge phase gathers h[src] rows with batched indirect DMA (edges are
    sorted by dst on the host and packed into 128-edge chunks per dst tile),
    and performs the segment softmax + scatter-add as one-hot matmuls on the
    PE: for each chunk, Sw[e,m] = (dstloc[e]==m) * exp(leakyrelu(s_src+s_dst))
    built in a single DVE tensor_scalar op; PSUM accumulates [128 dst, 129]
    where column 128 (driven by a constant-ones row column) is the softmax
    denominator.
  - Padding edges use src=dst=0 and dstloc=-1 so they contribute exactly 0.
"""

import sys

sys.path.insert(0, "/opt/trn_rl_repo")

import numpy as np
from contextlib import ExitStack

from concourse import bass, bacc, mybir
import concourse.tile as tile
from concourse.bass_utils import run_bass_kernel_spmd

F32 = mybir.dt.float32
I32 = mybir.dt.int32
OP = mybir.AluOpType

P = 128
D = 128
ROW = 136  # fp32 words per augmented row (544B, 32B aligned)
COL_ONES = 128
COL_SSRC = 129
COL_SDST = 130
NEG_SLOPE = 0.2
N_CORES = 8

N_FULL = 50000


def _prep_graph(edge_index, n_nodes, n_cores):
    """Sort edges (plus self loops) by dst, pack into per-tile 128-edge chunks.

    Returns (tiles_per_core, n_pad, n_chunks[tiles_per_core], metas[n_cores]).
    Each meta is an int32 1-D array: concatenated per-tile blocks [P, 3n]
    (src ids | dst ids | dstloc as f32 bits), row-major.
    """
    tiles_per_core = -(-n_nodes // (n_cores * P))
    n_pad = n_cores * tiles_per_core * P
    loops = np.arange(n_nodes, dtype=np.int64)
    src = np.concatenate([np.asarray(edge_index[0], dtype=np.int64), loops])
    dst = np.concatenate([np.asarray(edge_index[1], dtype=np.int64), loops])
    order = np.argsort(dst, kind="stable")
    src, dst = src[order], dst[order]

    n_tiles = n_cores * tiles_per_core
    counts = np.bincount(dst // P, minlength=n_tiles)
    starts = np.concatenate([[0], np.cumsum(counts)])

    n_chunks = []
    for s in range(tiles_per_core):
        m = 1
        for c in range(n_cores):
            m = max(m, -(-int(counts[c * tiles_per_core + s]) // P))
        n_chunks.append(m)

    metas = []
    for c in range(n_cores):
        parts = []
        for s in range(tiles_per_core):
            t = c * tiles_per_core + s
            n = n_chunks[s]
            e0, e1 = int(starts[t]), int(starts[t + 1])
            cnt = e1 - e0
            blk_src = np.zeros((P, n), dtype=np.int64)
            blk_dst = np.zeros((P, n), dtype=np.int64)
            blk_loc = np.full((P, n), -1.0, dtype=np.float32)
            idx = np.arange(cnt)
            pp, cc = idx % P, idx // P
            blk_src[pp, cc] = src[e0:e1]
            blk_dst[pp, cc] = dst[e0:e1]
            blk_loc[pp, cc] = (dst[e0:e1] - t * P).astype(np.float32)
            blk = np.concatenate(
                [
                    blk_src.astype(np.int32),
                    blk_dst.astype(np.int32),
                    blk_loc.view(np.int32),
                ],
                axis=1,
            )
            parts.append(blk.reshape(-1))
        metas.append(np.ascontiguousarray(np.concatenate(parts)))
    return tiles_per_core, n_pad, n_chunks, metas


def _aug(w, a_s, a_d):
    w = np.asarray(w, dtype=np.float32)
    return np.ascontiguousarray(
        np.concatenate(
            [w, (w @ np.asarray(a_s, np.float32))[:, None], (w @ np.asarray(a_d, np.float32))[:, None]],
            axis=1,
        ).astype(np.float32)
    )


def _build_program(tiles_per_core, n_chunks, n_cores, n_layers=4, debug_dump=False):
    npc = tiles_per_core * P
    n_pad = n_cores * npc
    meta_words = P * 3 * sum(n_chunks)

    nc = bacc.Bacc("TRN2", target_bir_lowering=False, debug=False, num_devices=n_cores)
    dbg_haug = dbg_g = None
    if debug_dump:
        dbg_haug = nc.dram_tensor("dbg_haug", [n_pad, ROW], F32, kind="ExternalOutput").ap()
        dbg_g = nc.dram_tensor("dbg_g", [P, n_chunks[0] * ROW], F32, kind="ExternalOutput").ap()
        dbg_ex = nc.dram_tensor("dbg_ex", [P, n_chunks[0]], F32, kind="ExternalOutput").ap()
        dbg_sw = nc.dram_tensor("dbg_sw", [P, P], F32, kind="ExternalOutput").ap()

    x_in = nc.dram_tensor("x_local", [npc, D], F32, kind="ExternalInput").ap()
    meta_in = nc.dram_tensor("meta", [meta_words], I32, kind="ExternalInput").ap()
    iota_in = nc.dram_tensor("iota", [P, P], F32, kind="ExternalInput").ap()
    ident_in = nc.dram_tensor("ident", [P, P], F32, kind="ExternalInput").ap()
    w_names = ["w_enc", "w_p1", "w_p2h", "w_p2e", "w_dec"]
    w_aps = [nc.dram_tensor(nm, [D, D + 2], F32, kind="ExternalInput").ap() for nm in w_names]
    b_aps = [nc.dram_tensor(nm, [P, D], F32, kind="ExternalInput").ap() for nm in ["b_enc", "b_p", "b_dec"]]
    y_out = nc.dram_tensor("y_out", [npc, D], F32, kind="ExternalOutput").ap()

    with ExitStack() as st:
        tc = st.enter_context(tile.TileContext(nc))
        cpool = st.enter_context(tc.tile_pool(name="consts", bufs=1))
        apool = st.enter_context(tc.tile_pool(name="pha", bufs=3))
        gpool = st.enter_context(tc.tile_pool(name="gat", bufs=3))
        swpool = st.enter_context(tc.tile_pool(name="sw", bufs=4))
        epool = st.enter_context(tc.tile_pool(name="epi", bufs=2))
        pp = st.enter_context(tc.tile_pool(name="ps", bufs=2, space="PSUM"))
        dpool = st.enter_context(tc.tile_pool(name="dramp", bufs=1, space="DRAM"))

        ag_in = dpool.tile([npc, ROW], F32, name="ag_in")
        haugs = [
            dpool.tile([n_pad, ROW], F32, addr_space="Shared", name=f"haug{i}")
            for i in range(4)
        ]
        y_mid = [dpool.tile([npc, D], F32, name=f"ymid{i}") for i in range(3)]

        iota_t = cpool.tile([P, P], F32, name="iota_t")
        nc.sync.dma_start(iota_t[:], iota_in)
        ident_t = cpool.tile([P, P], F32, name="ident_t")
        nc.sync.dma_start(ident_t[:], ident_in)
        w_t = []
        for i, ap in enumerate(w_aps):
            wt = cpool.tile([D, D + 2], F32, name=f"w_t{i}")
            nc.sync.dma_start(wt[:], ap)
            w_t.append(wt)
        b_t = []
        for i, ap in enumerate(b_aps):
            bt = cpool.tile([P, D], F32, name=f"b_t{i}")
            nc.sync.dma_start(bt[:], ap)
            b_t.append(bt)

        def phase_a(x_srcs, w_tiles):
            for s in range(tiles_per_core):
                r0 = s * P
                pa = pp.tile([P, D + 2], F32, tag="pa")
                for k, (x_src, wt) in enumerate(zip(x_srcs, w_tiles)):
                    xa = apool.tile([P, D], F32, tag="xa")
                    nc.sync.dma_start(xa[:], x_src[r0 : r0 + P, :])
                    pt = pp.tile([P, P], F32, tag="pt")
                    nc.tensor.transpose(pt[:], xa[:], ident_t[:])
                    xt = apool.tile([P, D], F32, tag="xt")
                    nc.vector.tensor_copy(xt[:], pt[:])
                    nc.tensor.matmul(
                        pa[:],
                        lhsT=xt[:],
                        rhs=wt[:],
                        start=(k == 0),
                        stop=(k == len(x_srcs) - 1),
                    )
                ob = apool.tile([P, ROW], F32, tag="ob")
                nc.vector.tensor_copy(ob[:, 0:D], pa[:, 0:D])
                nc.vector.memset(ob[:, COL_ONES : COL_ONES + 1], 1.0)
                nc.vector.tensor_copy(ob[:, COL_SSRC : COL_SDST + 1], pa[:, D : D + 2])
                nc.vector.memset(ob[:, COL_SDST + 1 : ROW], 0.0)
                nc.sync.dma_start(ag_in[r0 : r0 + P, :], ob[:])

        def phase_b(haug, y_dst, bt, dump=False):
            off_words = 0
            for s in range(tiles_per_core):
                n = n_chunks[s]
                mt = apool.tile([P, 3 * n], I32, tag="meta")
                nc.sync.dma_start(
                    mt[:],
                    meta_in[off_words : off_words + P * 3 * n].rearrange(
                        "(p w) -> p w", w=3 * n
                    ),
                )
                off_words += P * 3 * n
                locf = mt[:, 2 * n : 3 * n].bitcast(F32)
                pacc = pp.tile([P, D + 1], F32, tag="pacc")
                for c in range(n):
                    g = gpool.tile([P, ROW], F32, tag="G")
                    nc.gpsimd.indirect_dma_start(
                        out=g[:],
                        out_offset=None,
                        in_=haug[:],
                        in_offset=bass.IndirectOffsetOnAxis(ap=mt[:, c : c + 1], axis=0),
                    )
                    # in-flight CCE add: col SSRC becomes s_src[src] + s_dst[dst]
                    nc.gpsimd.indirect_dma_start(
                        out=g[:, COL_SSRC : COL_SSRC + 1],
                        out_offset=None,
                        in_=haug[:],
                        in_offset=bass.IndirectOffsetOnAxis(
                            ap=mt[:, n + c : n + c + 1], axis=0
                        ),
                        element_offset=COL_SDST,
                        compute_op=OP.add,
                    )
                    es = epool.tile([P, 1], F32, tag="es")
                    nc.vector.tensor_scalar(
                        es[:], g[:, COL_SSRC : COL_SSRC + 1], NEG_SLOPE, None, op0=OP.mult
                    )
                    el = epool.tile([P, 1], F32, tag="el")
                    nc.vector.tensor_tensor(
                        el[:], es[:], g[:, COL_SSRC : COL_SSRC + 1], op=OP.max
                    )
                    ex = epool.tile([P, 1], F32, tag="ex")
                    nc.scalar.activation(ex[:], el[:], mybir.ActivationFunctionType.Exp)
                    sw = swpool.tile([P, P], F32, tag="sw")
                    nc.vector.tensor_scalar(
                        sw[:],
                        iota_t[:],
                        locf[:, c : c + 1],
                        ex[:, 0:1],
                        op0=OP.is_equal,
                        op1=OP.mult,
                    )
                    nc.tensor.matmul(
                        pacc[:],
                        lhsT=sw[:],
                        rhs=g[:, 0 : D + 1],
                        start=(c == 0),
                        stop=(c == n - 1),
                    )
                den = epool.tile([P, 1], F32, tag="den")
                nc.vector.tensor_scalar(den[:], pacc[:, D : D + 1], 1e-30, None, op0=OP.add)
                rden = epool.tile([P, 1], F32, tag="rden")
                nc.vector.reciprocal(rden[:], den[:])
                ot = epool.tile([P, D], F32, tag="ot")
                nc.vector.tensor_scalar(ot[:], pacc[:, 0:D], rden[:, 0:1], None, op0=OP.mult)
                nc.vector.tensor_tensor(ot[:], ot[:], bt[:], op=OP.add)
                nc.sync.dma_start(y_dst[s * P : (s + 1) * P, :], ot[:])

        layers = [
            ([x_in], [w_t[0]], y_mid[0], b_t[0], haugs[0]),
            ([y_mid[0]], [w_t[1]], y_mid[1], b_t[1], haugs[1]),
            ([y_mid[1], y_mid[0]], [w_t[2], w_t[3]], y_mid[2], b_t[1], haugs[2]),
            ([y_mid[2]], [w_t[4]], y_out, b_t[2], haugs[3]),
        ]
        layers = layers[:n_layers]
        if n_layers < 4:
            srcs, wts, ydst, bt, hb = layers[-1]
            layers[-1] = (srcs, wts, y_out, bt, hb)
        for li, (srcs, wts, ydst, bt, hb) in enumerate(layers):
            phase_a(srcs, wts)
            nc.gpsimd.collective_compute(
                "AllGather",
                OP.bypass,
                replica_groups=[list(range(n_cores))],
                ins=[ag_in.opt()],
                outs=[hb.opt()],
            )
            phase_b(hb, ydst, bt)

    nc.compile()
    return nc


_CACHE = {}


def _get_compiled(edge_index, n_nodes, n_cores, n_layers=4, debug_dump=False):
    key = (n_nodes, n_cores, n_layers, debug_dump, hash(np.asarray(edge_index).tobytes()))
    if key not in _CACHE:
        tiles_per_core, n_pad, n_chunks, metas = _prep_graph(edge_index, n_nodes, n_cores)
        nc = _build_program(tiles_per_core, n_chunks, n_cores, n_layers, debug_dump)
        _CACHE.clear()
        _CACHE[key] = (nc, tiles_per_core, n_pad, metas)
    return _CACHE[key]


def _run(
    x,
    edge_index,
    We,
    ae_s,
    ae_d,
    be,
    Wp,
    ap_s,
    ap_d,
    bp,
    Wd,
    ad_s,
    ad_d,
    bd,
    n_nodes=N_FULL,
    n_cores=N_CORES,
    trace=False,
    n_layers=4,
    debug_dump=False,
):
    nc, tiles_per_core, n_pad, metas = _get_compiled(edge_index, n_nodes, n_cores, n_layers, debug_dump)
    npc = tiles_per_core * P

    x = np.asarray(x, dtype=np.float32)
    x_pad = np.zeros((n_pad, D), dtype=np.float32)
    x_pad[:n_nodes] = x

    Wp = np.asarray(Wp, dtype=np.float32)
    Wp1, Wp2 = Wp[:D], Wp[D:]
    w_vals = [
        _aug(We, ae_s, ae_d),
        _aug(Wp1 + Wp2, ap_s, ap_d),
        _aug(Wp1, ap_s, ap_d),
        _aug(Wp2, ap_s, ap_d),
        _aug(Wd, ad_s, ad_d),
    ]
    b_vals = [
        np.ascontiguousarray(np.broadcast_to(np.asarray(b, np.float32), (P, D)))
        for b in [be, bp, bd]
    ]
    iota_v = np.ascontiguousarray(
        np.broadcast_to(np.arange(P, dtype=np.float32), (P, P))
    )
    ident_v = np.eye(P, dtype=np.float32)

    in_maps = []
    for c in range(n_cores):
        m = {
            "x_local": np.ascontiguousarray(x_pad[c * npc : (c + 1) * npc]),
            "meta": metas[c],
            "iota": iota_v,
            "ident": ident_v,
            "w_enc": w_vals[0],
            "w_p1": w_vals[1],
            "w_p2h": w_vals[2],
            "w_p2e": w_vals[3],
            "w_dec": w_vals[4],
            "b_enc": b_vals[0],
            "b_p": b_vals[1],
            "b_dec": b_vals[2],
        }
        in_maps.append(m)

    res = run_bass_kernel_spmd(
        nc, in_maps, core_ids=list(range(n_cores)), trace=trace
    )
    out = np.concatenate([res.results[c]["y_out"] for c in range(n_cores)], axis=0)
    return out[:n_nodes].astype(np.float32), res


def kernel(**inputs):
    out, _ = _run(**inputs)
    return out


def kernel_traced(**inputs):
    out, res = _run(**inputs, trace=True)
    return out, res
